# revision 33
# baseline (speedup 1.0000x reference)
"""GRU unit kernel for Trainium2, data-parallel over 8 NeuronCores.

Computation (per batch row):
    r  = sigmoid(x @ W_i2r + b_i2r + h @ W_h2r)
    z  = sigmoid(x @ W_i2z + b_i2z + h @ W_h2z)
    h1 = tanh   (x @ W_i2h + b_i2h + r * (h @ W_h2h))
    out = (1 - z) * h1 + z * hidden

Sharding: batch (16384) split 8 ways; weights replicated.

Mixed precision: the r-gate matmuls (x@W_i2r, h@W_h2r) and h@W_h2h run as
fp8e4 DoubleRow matmuls (2 MACs/cell/cycle, K=256 per pass); the z-gate and
x@W_i2h stay bf16 (their quantization error is amplified the most by the
output blend, measured on the full batch). fp8 operands are pre-scaled on
the host (x,h by 16; W by 64 — lifts the uniform(-1/32,1/32) weights out of
e4m3's subnormal range); the resulting 2^10 factor is removed by the
activation instruction's scale parameter in the epilogue.

Device kernel per core (B_local=2048 rows = 16 m-tiles of 128):
  - weights + x/h transposed operand tiles resident in SBUF, h (f32) and
    out streamed.
  - per m-tile: 24 DoubleRow fp8 + 48 bf16 matmuls of N=512 into 8 PSUM
    banks (r/z/a/b gates x two 512-halves), fp32 accumulation; epilogue on
    DVE (bias adds, blend) + ACT (sigmoid/tanh with 2^-10 scale).
"""

import os
import numpy as np
import ml_dtypes
from contextlib import ExitStack

import concourse.bass as bass
import concourse.tile as tile
from concourse import bacc, mybir

N_CORES = 8
B, I, H = 16384, 1024, 1024
BL = B // N_CORES           # 2048 batch rows per core
MT = BL // 128              # 16 m-tiles
KO = I // 128               # 8 k-tiles of 128
KP = KO // 2                # 4 fp8 DoubleRow k-pair-tiles
F32 = mybir.dt.float32
BF16 = mybir.dt.bfloat16
FP8 = mybir.dt.float8e4
BF16_NP = ml_dtypes.bfloat16
FP8_NP = ml_dtypes.float8_e4m3
ASCALE = 16.0               # fp8 activation pre-scale
WSCALE = 64.0               # fp8 weight pre-scale
PSCALE = ASCALE * WSCALE    # fp8-path PSUM scale (2^10)
DR = mybir.MatmulPerfMode.DoubleRow


def _ap_key(a):
    try:
        return (a.memref, a.offset, str(a.ap), str(a.dtype))
    except Exception:
        return ("?", id(a))


def dedupe_ldweights(nc):
    """Drop InstLdweights that reload the stationary tile already resident in
    the PE array (bacc emits one per matmul). The paired InstMatmult keeps
    both APs, so data deps survive; the removed LDW's scheduling deps are
    merged into the following instruction."""
    total_removed = 0
    for blk in nc.m.functions[0].blocks:
        insts = list(blk.instructions)
        new = []
        last_key = None
        pending = []
        for i in insts:
            t = type(i).__name__
            eng = str(getattr(i, "engine", ""))
            if t == "InstLdweights":
                key = (_ap_key(i.ins[0]), str(i.perf_mode),
                       str(i.tile_position), str(i.is_transpose))
                if key == last_key:
                    pending.append(i)
                    total_removed += 1
                    continue
                last_key = key
                new.append(i)
            else:
                if "PE" in eng and t not in ("InstMatmult",
                                             "InstEventSemaphore"):
                    last_key = None  # unknown PE inst may clobber weights
                if pending and t == "InstMatmult":
                    for j in pending:
                        i.merge_dependencies_from(j)
                    pending = []
                new.append(i)
        if pending:
            new.extend(pending)
        blk.instructions = new
    return total_removed


def build_nc(reps: int = 1):
    nc = bacc.Bacc("TRN2", target_bir_lowering=False, debug=False,
                   num_devices=N_CORES)
    AF = mybir.ActivationFunctionType

    # All bulk HBM tensors are pre-arranged on the host into the exact SBUF
    # layout (partition dim = ki second), so every transfer is one fully
    # contiguous run per partition: activations chunk-major [CH][ki][ko][cw],
    # weights gate-major [3][ki][ko][H].
    CH = 8
    cw = BL // CH
    # fp8 operands: x/h transposed, scaled by 16; W_i2r|W_h2r|W_h2h scaled 64
    xt8 = nc.dram_tensor("xt8", [CH, 128, KO, cw], FP8, kind="ExternalInput").ap()
    ht8 = nc.dram_tensor("ht8", [CH, 128, KO, cw], FP8, kind="ExternalInput").ap()
    w8 = nc.dram_tensor("w8", [3, 128, KO, H], FP8, kind="ExternalInput").ap()
    # bf16 operands: x/h transposed; W_i2z|W_h2z|W_i2h*1024
    xtb = nc.dram_tensor("xtb", [CH, 128, KO, cw], BF16, kind="ExternalInput").ap()
    htb = nc.dram_tensor("htb", [CH, 128, KO, cw], BF16, kind="ExternalInput").ap()
    wb = nc.dram_tensor("wb", [3, 128, KO, H], BF16, kind="ExternalInput").ap()
    # biases pre-broadcast to 128 partitions: b_r*1024 | b_z | b_a*1024
    bias = nc.dram_tensor("bias", [128, 3 * H], F32, kind="ExternalInput").ap()
    hbl = nc.dram_tensor("hbl", [BL, H], BF16, kind="ExternalInput").ap()
    out = nc.dram_tensor("out", [BL, H], BF16, kind="ExternalOutput").ap()

    with tile.TileContext(nc) as tc, ExitStack() as ctx:
        wpool = ctx.enter_context(tc.tile_pool(name="w", bufs=1))
        apool = ctx.enter_context(tc.tile_pool(name="a", bufs=1))
        hpool = ctx.enter_context(tc.tile_pool(name="h", bufs=2))
        epool = ctx.enter_context(tc.tile_pool(name="e", bufs=2))
        psum = ctx.enter_context(tc.tile_pool(name="ps", bufs=1, space="PSUM"))

        w8_sb = wpool.tile([128, 3, KO, H], FP8, tag="w8")
        wb_sb = wpool.tile([128, 3, KO, H], BF16, tag="wb")
        bias_sb = wpool.tile([128, 3 * H], F32, tag="bias")
        xt8_sb = apool.tile([128, CH, KO, cw], FP8, tag="xt8")
        ht8_sb = apool.tile([128, CH, KO, cw], FP8, tag="ht8")
        xtb_sb = apool.tile([128, CH, KO, cw], BF16, tag="xtb")
        htb_sb = apool.tile([128, CH, KO, cw], BF16, tag="htb")

        def body():
            emit_loads()
            for mt in range(MT):
                emit_mtile(mt)

        def emit_loads():
            # DMA order = first-use order for m-tile 0's phases:
            # x-fp8[r], h-fp8[r,b], x-bf16[z,a], h-bf16[z].
            # Queue order is also tuned for the reps-loop steady state:
            # entries are consumed FIFO, and an entry can only start once its
            # SBUF destination's last reader (previous iteration) is done.
            # The chunk-0 activations free early; the weights free in m-tile
            # 15, phase by phase, in exactly this order; the later activation
            # chunks free progressively and stream during the iteration.
            nc.gpsimd.dma_start(bias_sb[:], bias)
            nc.sync.dma_start(xt8_sb[:, 0], xt8[0])
            nc.sync.dma_start(w8_sb[:, 0], w8[0])
            nc.sync.dma_start(ht8_sb[:, 0], ht8[0])
            nc.sync.dma_start(w8_sb[:, 1], w8[1])
            nc.sync.dma_start(w8_sb[:, 2], w8[2])
            nc.sync.dma_start(xtb_sb[:, 0], xtb[0])
            nc.sync.dma_start(wb_sb[:, 0], wb[0])
            nc.sync.dma_start(wb_sb[:, 2], wb[2])
            nc.sync.dma_start(htb_sb[:, 0], htb[0])
            nc.sync.dma_start(wb_sb[:, 1], wb[1])
            for c in range(1, CH):
                nc.sync.dma_start(xt8_sb[:, c], xt8[c])
                nc.sync.dma_start(ht8_sb[:, c], ht8[c])
                nc.sync.dma_start(xtb_sb[:, c], xtb[c])
                nc.sync.dma_start(htb_sb[:, c], htb[c])

        def emit_mtile(mt):
            mpc = cw // 128            # m-tiles per activation chunk
            c, m0 = mt // mpc, (mt % mpc) * 128
            ms = slice(m0, m0 + 128)
            hb_t = hpool.tile([128, H], BF16, tag="hbl")
            nc.gpsimd.dma_start(hb_t[:], hbl[mt * 128:(mt + 1) * 128, :])

            ps = {}
            for g in ("r", "z", "a", "b"):
                for nh in range(2):
                    ps[(g, nh)] = psum.tile([128, 512], F32, tag=f"p{g}{nh}",
                                            name=f"p{g}{nh}")

            def x_fp8():
                # r-gate x side: DoubleRow over ko-pairs, w8 gate 0
                for kp in range(KP):
                    kk = slice(2 * kp, 2 * kp + 2)
                    for nh in range(2):
                        o = nh * 512
                        nc.tensor.matmul(ps[("r", nh)], xt8_sb[:, c, kk, ms],
                                         w8_sb[:, 0, kk, o:o + 512],
                                         start=(kp == 0), stop=False,
                                         perf_mode=DR)

            def x_bf16():
                # z (wb gate 0) and a (wb gate 2) gates
                for ko in range(KO):
                    for nh in range(2):
                        o = nh * 512
                        nc.tensor.matmul(ps[("z", nh)], xtb_sb[:, c, ko, ms],
                                         wb_sb[:, 0, ko, o:o + 512],
                                         start=(ko == 0), stop=False)
                        nc.tensor.matmul(ps[("a", nh)], xtb_sb[:, c, ko, ms],
                                         wb_sb[:, 2, ko, o:o + 512],
                                         start=(ko == 0), stop=(ko == KO - 1))

            def h_bf16():
                # z-gate h side: wb gate 1
                for ko in range(KO):
                    for nh in range(2):
                        o = nh * 512
                        nc.tensor.matmul(ps[("z", nh)], htb_sb[:, c, ko, ms],
                                         wb_sb[:, 1, ko, o:o + 512],
                                         start=False, stop=(ko == KO - 1))

            def h_fp8():
                # r-gate h side (w8 gate 1) + b = h@W_h2h (w8 gate 2)
                for kp in range(KP):
                    kk = slice(2 * kp, 2 * kp + 2)
                    for nh in range(2):
                        o = nh * 512
                        nc.tensor.matmul(ps[("r", nh)], ht8_sb[:, c, kk, ms],
                                         w8_sb[:, 1, kk, o:o + 512],
                                         start=False, stop=(kp == KP - 1),
                                         perf_mode=DR)
                        nc.tensor.matmul(ps[("b", nh)], ht8_sb[:, c, kk, ms],
                                         w8_sb[:, 2, kk, o:o + 512],
                                         start=(kp == 0), stop=(kp == KP - 1),
                                         perf_mode=DR)

            # Uniform phase order, fp8 before bf16: (1) at startup the small
            # fp8 operands land first; (2) r/b finish mid-m-tile, so the
            # epilogue's long chain (r*b -> tanh) overlaps the z/a matmuls
            # and the r/b banks free before the next m-tile needs them;
            # (3) in the reps loop, weights free in exactly the order the
            # next iteration's reloads sit in the DMA queue.
            x_fp8(); h_fp8(); x_bf16(); h_bf16()

            def epilogue(o, w):
                nh, po = o // 512, o % 512
                psl = slice(po, po + w)
                pr, pz = ps[("r", nh)], ps[("z", nh)]
                pa, pb = ps[("a", nh)], ps[("b", nh)]
                tr = epool.tile([128, w], F32, tag=f"tr{w}")
                ta = epool.tile([128, w], F32, tag=f"ta{w}")
                h1 = epool.tile([128, w], BF16, tag=f"h1{w}")
                tz = epool.tile([128, w], BF16, tag=f"tz{w}")
                td = epool.tile([128, w], BF16, tag=f"td{w}")
                nc.vector.tensor_add(tr[:], pr[:, psl], bias_sb[:, o:o + w])
                nc.scalar.activation(tr[:], tr[:], AF.Sigmoid,
                                     scale=1.0 / PSCALE)            # r
                nc.vector.tensor_mul(ta[:], tr[:], pb[:, psl])         # r*(hU)
                nc.vector.tensor_add(ta[:], ta[:], pa[:, psl])
                nc.vector.tensor_add(ta[:], ta[:],
                                     bias_sb[:, 2 * H + o:2 * H + o + w])
                nc.scalar.activation(h1[:], ta[:], AF.Tanh,
                                     scale=1.0 / PSCALE)            # h1 (bf16)
                nc.vector.tensor_add(tr[:], pz[:, psl], bias_sb[:, H + o:H + o + w])
                nc.scalar.activation(tz[:], tr[:], AF.Sigmoid)      # z (bf16)
                nc.vector.tensor_sub(td[:], hb_t[:, o:o + w], h1[:])
                nc.vector.tensor_mul(td[:], tz[:], td[:])           # z*(h-h1)
                nc.vector.tensor_add(tz[:], h1[:], td[:])           # out
                gms = slice(mt * 128, (mt + 1) * 128)
                nc.scalar.dma_start(out[gms, o:o + w], tz[:])

            epilogue(0, 512); epilogue(512, 512)

        if reps > 1:
            with tc.For_i(0, reps, 1):
                body()
        else:
            body()

    nc.compile()
    if os.environ.get("GRU_DEDUP", "1") == "1":
        dedupe_ldweights(nc)
    return nc


def _chunk_major(aT, n_chunks):
    """[I, BLc] (transposed activations) -> [CH, 128ki, KO, cw] contiguous,
    i.e. the SBUF layout, so each chunk DMA is one contiguous run per
    partition."""
    blc = aT.shape[1]
    cwc = blc // n_chunks
    # [ (ko ki), (c b) ] -> [c, ki, ko, b]
    v = aT.reshape(KO, 128, n_chunks, cwc).transpose(2, 1, 0, 3)
    return np.ascontiguousarray(v)


def _gate_major(ws):
    """list of 3 [I, H] weight mats -> [3, 128ki, KO, H] (SBUF layout)."""
    v = np.stack([w.reshape(KO, 128, H).transpose(1, 0, 2) for w in ws])
    return np.ascontiguousarray(v)


def prep_in_maps(inputs):
    """Host-side marshalling: shard batch, transpose+cast activations,
    concat+scale weights/biases into SBUF-layout arrays. Returns per-core
    input dicts."""
    g = {k: np.asarray(v) for k, v in inputs.items()}
    x, h = g["inputs"], g["hidden"]
    w8 = _gate_major([
        (g["W_i2r"] * WSCALE).astype(FP8_NP),
        (g["W_h2r"] * WSCALE).astype(FP8_NP),
        (g["W_h2h"] * WSCALE).astype(FP8_NP),
    ])
    wb = _gate_major([
        g["W_i2z"].astype(BF16_NP),
        g["W_h2z"].astype(BF16_NP),
        (g["W_i2h"] * PSCALE).astype(BF16_NP),
    ])
    b = np.concatenate([g["b_i2r"] * PSCALE, g["b_i2z"],
                        g["b_i2h"] * PSCALE]).astype(np.float32)
    bias_b = np.ascontiguousarray(np.broadcast_to(b, (128, 3 * H)))
    xt8_all = np.ascontiguousarray(x.T * ASCALE).astype(FP8_NP)
    ht8_all = np.ascontiguousarray(h.T * ASCALE).astype(FP8_NP)
    xtb_all = x.T.astype(BF16_NP, order="C")
    htb_all = h.T.astype(BF16_NP, order="C")
    CH = 8
    in_maps = []
    for c in range(N_CORES):
        sl = slice(c * BL, (c + 1) * BL)
        in_maps.append({
            "xt8": _chunk_major(xt8_all[:, sl], CH),
            "ht8": _chunk_major(ht8_all[:, sl], CH),
            "xtb": _chunk_major(xtb_all[:, sl], CH),
            "htb": _chunk_major(htb_all[:, sl], CH),
            "hbl": np.ascontiguousarray(h[sl].astype(BF16_NP)),
            "w8": w8,
            "wb": wb,
            "bias": bias_b,
        })
    return in_maps


_RUNNERS = {}


def get_runner(reps: int = 1):
    """Build the bass module once and wrap it in a jitted 8-way shard_map,
    mirroring concourse.bass2jax.run_bass_via_pjrt but reusable across calls
    (so repeated executions don't re-trace/re-compile). reps>1 wraps the
    whole kernel in an on-device loop (for timing via amortization)."""
    if reps in _RUNNERS:
        return _RUNNERS[reps]
    import jax
    from jax.sharding import Mesh, PartitionSpec
    from jax.experimental.shard_map import shard_map
    from concourse.bass2jax import (_bass_exec_p, install_neuronx_cc_hook,
                                    partition_id_tensor)

    nc = build_nc(reps)
    install_neuronx_cc_hook()

    partition_name = (nc.partition_id_tensor.name
                      if nc.partition_id_tensor else None)
    in_names, out_names, out_avals, zero_outs = [], [], [], []
    for alloc in nc.m.functions[0].allocations:
        if not isinstance(alloc, mybir.MemoryLocationSet):
            continue
        name = alloc.memorylocations[0].name
        if alloc.kind == "ExternalInput":
            if name != partition_name:
                in_names.append(name)
        elif alloc.kind == "ExternalOutput":
            out_names.append(name)
            shape = tuple(alloc.tensor_shape)
            dtype = mybir.dt.np(alloc.dtype)
            out_avals.append(jax.core.ShapedArray(shape, dtype))
            zero_outs.append(np.zeros(shape, dtype))
    all_names = in_names + out_names
    if partition_name is not None:
        all_names = all_names + [partition_name]
    all_names = tuple(all_names)
    n_in, n_out = len(in_names), len(out_names)

    def _body(*args):
        operands = list(args)
        if partition_name is not None:
            operands.append(partition_id_tensor())
        outs = _bass_exec_p.bind(
            *operands,
            out_avals=tuple(out_avals),
            in_names=all_names,
            out_names=tuple(out_names),
            lowering_input_output_aliases=(),
            sim_require_finite=True,
            sim_require_nnan=True,
            nc=nc,
        )
        return tuple(outs)

    devices = jax.devices()[:N_CORES]
    mesh = Mesh(np.asarray(devices), ("core",))
    sharded = jax.jit(
        shard_map(_body, mesh=mesh,
                  in_specs=(PartitionSpec("core"),) * (n_in + n_out),
                  out_specs=(PartitionSpec("core"),) * n_out,
                  check_rep=False),
        donate_argnums=tuple(range(n_in, n_in + n_out)),
        keep_unused=True,
    )
    _RUNNERS[reps] = (sharded, in_names, out_names, zero_outs)
    return _RUNNERS[reps]


def run_on_device(in_maps):
    sharded, in_names, out_names, zero_outs = get_runner()
    concat_in = [np.concatenate([m[n] for m in in_maps], axis=0)
                 for n in in_names]
    concat_zero = [np.zeros((N_CORES * z.shape[0], *z.shape[1:]), z.dtype)
                   for z in zero_outs]
    outs = sharded(*concat_in, *concat_zero)
    return {n: np.asarray(o) for n, o in zip(out_names, outs)}


_NC = None


def kernel(**inputs):
    """Full-input entry point: shard, run on 8 NeuronCores, gather."""
    global _NC
    from concourse._compat import axon_active
    in_maps = prep_in_maps(inputs)
    if axon_active():
        return run_on_device(in_maps)["out"].astype(np.float32)
    from concourse.bass_utils import run_bass_kernel_spmd
    if _NC is None:
        _NC = build_nc(1)
    res = run_bass_kernel_spmd(_NC, in_maps, core_ids=list(range(N_CORES)))
    return np.concatenate([res.results[c]["out"] for c in range(N_CORES)],
                          axis=0).astype(np.float32)


# revision 34
# speedup vs baseline: 1.1246x; 1.1246x over previous
"""GRU unit kernel for Trainium2, data-parallel over 8 NeuronCores.

Computation (per batch row):
    r  = sigmoid(x @ W_i2r + b_i2r + h @ W_h2r)
    z  = sigmoid(x @ W_i2z + b_i2z + h @ W_h2z)
    h1 = tanh   (x @ W_i2h + b_i2h + r * (h @ W_h2h))
    out = (1 - z) * h1 + z * hidden

Sharding: batch (16384) split 8 ways; weights replicated.

Mixed precision: the r-gate matmuls (x@W_i2r, h@W_h2r) and h@W_h2h run as
fp8e4 DoubleRow matmuls (2 MACs/cell/cycle, K=256 per pass); the z-gate and
x@W_i2h stay bf16 (their quantization error is amplified the most by the
output blend, measured on the full batch). fp8 operands are pre-scaled on
the host (x,h by 16; W by 64 — lifts the uniform(-1/32,1/32) weights out of
e4m3's subnormal range); the resulting 2^10 factor is removed by the
activation instruction's scale parameter in the epilogue.

Device kernel per core (B_local=2048 rows = 16 m-tiles of 128):
  - weights + x/h transposed operand tiles resident in SBUF (HBM arrays are
    pre-arranged host-side into the exact SBUF layout so every bulk DMA is
    one contiguous run per partition); h (bf16) and out (bf16) streamed on
    separate DMA queues.
  - per m-tile: 24 DoubleRow fp8 + 48 bf16 matmuls of N=512 into 8 PSUM
    banks (r/z/a/b gates x two 512-halves), fp32 accumulation; epilogue on
    DVE (bias adds, blend in bf16) + ACT (sigmoid/tanh with 2^-10 scale).
  - phase order fp8->bf16 per m-tile plus a DMA queue order chosen so the
    reps-loop steady state reloads weights exactly as m-tile 15 frees them.
"""

import os
import numpy as np
import ml_dtypes
from contextlib import ExitStack

import concourse.bass as bass
import concourse.tile as tile
from concourse import bacc, mybir

N_CORES = 8
B, I, H = 16384, 1024, 1024
BL = B // N_CORES           # 2048 batch rows per core
MT = BL // 128              # 16 m-tiles
KO = I // 128               # 8 k-tiles of 128
KP = KO // 2                # 4 fp8 DoubleRow k-pair-tiles
F32 = mybir.dt.float32
BF16 = mybir.dt.bfloat16
FP8 = mybir.dt.float8e4
BF16_NP = ml_dtypes.bfloat16
FP8_NP = ml_dtypes.float8_e4m3
ASCALE = 16.0               # fp8 activation pre-scale
WSCALE = 64.0               # fp8 weight pre-scale
PSCALE = ASCALE * WSCALE    # fp8-path PSUM scale (2^10)
DR = mybir.MatmulPerfMode.DoubleRow


def _ap_key(a):
    try:
        return (a.memref, a.offset, str(a.ap), str(a.dtype))
    except Exception:
        return ("?", id(a))


def dedupe_ldweights(nc):
    """Drop InstLdweights that reload the stationary tile already resident in
    the PE array (bacc emits one per matmul). The paired InstMatmult keeps
    both APs, so data deps survive; the removed LDW's scheduling deps are
    merged into the following instruction."""
    total_removed = 0
    for blk in nc.m.functions[0].blocks:
        insts = list(blk.instructions)
        new = []
        last_key = None
        pending = []
        for i in insts:
            t = type(i).__name__
            eng = str(getattr(i, "engine", ""))
            if t == "InstLdweights":
                key = (_ap_key(i.ins[0]), str(i.perf_mode),
                       str(i.tile_position), str(i.is_transpose))
                if key == last_key:
                    pending.append(i)
                    total_removed += 1
                    continue
                last_key = key
                new.append(i)
            else:
                if "PE" in eng and t not in ("InstMatmult",
                                             "InstEventSemaphore"):
                    last_key = None  # unknown PE inst may clobber weights
                if pending and t == "InstMatmult":
                    for j in pending:
                        i.merge_dependencies_from(j)
                    pending = []
                new.append(i)
        if pending:
            new.extend(pending)
        blk.instructions = new
    return total_removed


def build_nc(reps: int = 1):
    nc = bacc.Bacc("TRN2", target_bir_lowering=False, debug=False,
                   num_devices=N_CORES)
    AF = mybir.ActivationFunctionType

    # All bulk HBM tensors are pre-arranged on the host into the exact SBUF
    # layout (partition dim = ki second), so every transfer is one fully
    # contiguous run per partition: activations chunk-major [CH][ki][ko][cw],
    # weights gate-major [3][ki][ko][H].
    CH = 8
    cw = BL // CH
    # fp8 operands: x/h transposed, scaled by 16; W_i2r|W_h2r|W_h2h scaled 64
    xt8 = nc.dram_tensor("xt8", [CH, 128, KO, cw], FP8, kind="ExternalInput").ap()
    ht8 = nc.dram_tensor("ht8", [CH, 128, KO, cw], FP8, kind="ExternalInput").ap()
    w8 = nc.dram_tensor("w8", [3, 128, KO, H], FP8, kind="ExternalInput").ap()
    # bf16 operands: x/h transposed; W_i2z|W_h2z|W_i2h*1024
    xtb = nc.dram_tensor("xtb", [CH, 128, KO, cw], BF16, kind="ExternalInput").ap()
    htb = nc.dram_tensor("htb", [CH, 128, KO, cw], BF16, kind="ExternalInput").ap()
    wb = nc.dram_tensor("wb", [3, 128, KO, H], BF16, kind="ExternalInput").ap()
    # biases pre-broadcast to 128 partitions: b_r*1024 | b_z | b_a*1024
    bias = nc.dram_tensor("bias", [128, 3 * H], F32, kind="ExternalInput").ap()
    hbl = nc.dram_tensor("hbl", [BL, H], BF16, kind="ExternalInput").ap()
    out = nc.dram_tensor("out", [BL, H], BF16, kind="ExternalOutput").ap()

    with tile.TileContext(nc) as tc, ExitStack() as ctx:
        wpool = ctx.enter_context(tc.tile_pool(name="w", bufs=1))
        apool = ctx.enter_context(tc.tile_pool(name="a", bufs=1))
        hpool = ctx.enter_context(tc.tile_pool(name="h", bufs=2))
        epool = ctx.enter_context(tc.tile_pool(name="e", bufs=2))
        psum = ctx.enter_context(tc.tile_pool(name="ps", bufs=1, space="PSUM"))

        w8_sb = wpool.tile([128, 3, KO, H], FP8, tag="w8")
        wb_sb = wpool.tile([128, 3, KO, H], BF16, tag="wb")
        bias_sb = wpool.tile([128, 3 * H], F32, tag="bias")
        xt8_sb = apool.tile([128, CH, KO, cw], FP8, tag="xt8")
        ht8_sb = apool.tile([128, CH, KO, cw], FP8, tag="ht8")
        xtb_sb = apool.tile([128, CH, KO, cw], BF16, tag="xtb")
        htb_sb = apool.tile([128, CH, KO, cw], BF16, tag="htb")

        def body():
            emit_loads()
            for mt in range(MT):
                emit_mtile(mt)

        def emit_loads():
            # DMA order = first-use order for m-tile 0's phases:
            # x-fp8[r], h-fp8[r,b], x-bf16[z,a], h-bf16[z].
            # Queue order is also tuned for the reps-loop steady state:
            # entries are consumed FIFO, and an entry can only start once its
            # SBUF destination's last reader (previous iteration) is done.
            # The chunk-0 activations free early; the weights free in m-tile
            # 15, phase by phase, in exactly this order; the later activation
            # chunks free progressively and stream during the iteration.
            nc.gpsimd.dma_start(bias_sb[:], bias)
            nc.sync.dma_start(xt8_sb[:, 0], xt8[0])
            nc.sync.dma_start(w8_sb[:, 0], w8[0])
            nc.sync.dma_start(ht8_sb[:, 0], ht8[0])
            nc.sync.dma_start(w8_sb[:, 1], w8[1])
            nc.sync.dma_start(w8_sb[:, 2], w8[2])
            nc.sync.dma_start(xtb_sb[:, 0], xtb[0])
            nc.sync.dma_start(wb_sb[:, 0], wb[0])
            nc.sync.dma_start(wb_sb[:, 2], wb[2])
            nc.sync.dma_start(htb_sb[:, 0], htb[0])
            nc.sync.dma_start(wb_sb[:, 1], wb[1])
            for c in range(1, CH):
                nc.sync.dma_start(xt8_sb[:, c], xt8[c])
                nc.sync.dma_start(ht8_sb[:, c], ht8[c])
                nc.sync.dma_start(xtb_sb[:, c], xtb[c])
                nc.sync.dma_start(htb_sb[:, c], htb[c])

        def emit_mtile(mt):
            mpc = cw // 128            # m-tiles per activation chunk
            c, m0 = mt // mpc, (mt % mpc) * 128
            ms = slice(m0, m0 + 128)
            hb_t = hpool.tile([128, H], BF16, tag="hbl")
            nc.gpsimd.dma_start(hb_t[:], hbl[mt * 128:(mt + 1) * 128, :])

            ps = {}
            for g in ("r", "z", "a", "b"):
                for nh in range(2):
                    ps[(g, nh)] = psum.tile([128, 512], F32, tag=f"p{g}{nh}",
                                            name=f"p{g}{nh}")

            def x_fp8():
                # r-gate x side: DoubleRow over ko-pairs, w8 gate 0
                for kp in range(KP):
                    kk = slice(2 * kp, 2 * kp + 2)
                    for nh in range(2):
                        o = nh * 512
                        nc.tensor.matmul(ps[("r", nh)], xt8_sb[:, c, kk, ms],
                                         w8_sb[:, 0, kk, o:o + 512],
                                         start=(kp == 0), stop=False,
                                         perf_mode=DR)

            def x_bf16():
                # z (wb gate 0) and a (wb gate 2) gates
                for ko in range(KO):
                    for nh in range(2):
                        o = nh * 512
                        nc.tensor.matmul(ps[("z", nh)], xtb_sb[:, c, ko, ms],
                                         wb_sb[:, 0, ko, o:o + 512],
                                         start=(ko == 0), stop=False)
                        nc.tensor.matmul(ps[("a", nh)], xtb_sb[:, c, ko, ms],
                                         wb_sb[:, 2, ko, o:o + 512],
                                         start=(ko == 0), stop=(ko == KO - 1))

            def h_bf16():
                # z-gate h side: wb gate 1
                for ko in range(KO):
                    for nh in range(2):
                        o = nh * 512
                        nc.tensor.matmul(ps[("z", nh)], htb_sb[:, c, ko, ms],
                                         wb_sb[:, 1, ko, o:o + 512],
                                         start=False, stop=(ko == KO - 1))

            def h_fp8():
                # r-gate h side (w8 gate 1) + b = h@W_h2h (w8 gate 2)
                for kp in range(KP):
                    kk = slice(2 * kp, 2 * kp + 2)
                    for nh in range(2):
                        o = nh * 512
                        nc.tensor.matmul(ps[("r", nh)], ht8_sb[:, c, kk, ms],
                                         w8_sb[:, 1, kk, o:o + 512],
                                         start=False, stop=(kp == KP - 1),
                                         perf_mode=DR)
                        nc.tensor.matmul(ps[("b", nh)], ht8_sb[:, c, kk, ms],
                                         w8_sb[:, 2, kk, o:o + 512],
                                         start=(kp == 0), stop=(kp == KP - 1),
                                         perf_mode=DR)

            # Uniform phase order, fp8 before bf16: (1) at startup the small
            # fp8 operands land first; (2) r/b finish mid-m-tile, so the
            # epilogue's long chain (r*b -> tanh) overlaps the z/a matmuls
            # and the r/b banks free before the next m-tile needs them;
            # (3) in the reps loop, weights free in exactly the order the
            # next iteration's reloads sit in the DMA queue.
            x_fp8(); h_fp8(); x_bf16(); h_bf16()

            def epilogue(o, w):
                nh, po = o // 512, o % 512
                psl = slice(po, po + w)
                pr, pz = ps[("r", nh)], ps[("z", nh)]
                pa, pb = ps[("a", nh)], ps[("b", nh)]
                tr = epool.tile([128, w], F32, tag=f"tr{w}")
                ta = epool.tile([128, w], F32, tag=f"ta{w}")
                h1 = epool.tile([128, w], BF16, tag=f"h1{w}")
                tz = epool.tile([128, w], BF16, tag=f"tz{w}")
                td = epool.tile([128, w], BF16, tag=f"td{w}")
                nc.vector.tensor_add(tr[:], pr[:, psl], bias_sb[:, o:o + w])
                nc.scalar.activation(tr[:], tr[:], AF.Sigmoid,
                                     scale=1.0 / PSCALE)            # r
                nc.vector.tensor_mul(ta[:], tr[:], pb[:, psl])         # r*(hU)
                nc.vector.tensor_add(ta[:], ta[:], pa[:, psl])
                nc.vector.tensor_add(ta[:], ta[:],
                                     bias_sb[:, 2 * H + o:2 * H + o + w])
                nc.scalar.activation(h1[:], ta[:], AF.Tanh,
                                     scale=1.0 / PSCALE)            # h1 (bf16)
                nc.vector.tensor_add(tr[:], pz[:, psl], bias_sb[:, H + o:H + o + w])
                nc.scalar.activation(tz[:], tr[:], AF.Sigmoid)      # z (bf16)
                nc.vector.tensor_sub(td[:], hb_t[:, o:o + w], h1[:])
                nc.vector.tensor_mul(td[:], tz[:], td[:])           # z*(h-h1)
                nc.vector.tensor_add(tz[:], h1[:], td[:])           # out
                gms = slice(mt * 128, (mt + 1) * 128)
                nc.scalar.dma_start(out[gms, o:o + w], tz[:])

            epilogue(0, 512); epilogue(512, 512)

        if reps > 1:
            with tc.For_i(0, reps, 1):
                body()
        else:
            body()

    nc.compile()
    if os.environ.get("GRU_DEDUP", "1") == "1":
        dedupe_ldweights(nc)
    return nc


def _chunk_major(aT, n_chunks):
    """[I, BLc] (transposed activations) -> [CH, 128ki, KO, cw] contiguous,
    i.e. the SBUF layout, so each chunk DMA is one contiguous run per
    partition."""
    blc = aT.shape[1]
    cwc = blc // n_chunks
    # [ (ko ki), (c b) ] -> [c, ki, ko, b]
    v = aT.reshape(KO, 128, n_chunks, cwc).transpose(2, 1, 0, 3)
    return np.ascontiguousarray(v)


def _gate_major(ws):
    """list of 3 [I, H] weight mats -> [3, 128ki, KO, H] (SBUF layout)."""
    v = np.stack([w.reshape(KO, 128, H).transpose(1, 0, 2) for w in ws])
    return np.ascontiguousarray(v)


def prep_in_maps(inputs):
    """Host-side marshalling: shard batch, transpose+cast activations,
    concat+scale weights/biases into SBUF-layout arrays. Returns per-core
    input dicts."""
    g = {k: np.asarray(v) for k, v in inputs.items()}
    x, h = g["inputs"], g["hidden"]
    w8 = _gate_major([
        (g["W_i2r"] * WSCALE).astype(FP8_NP),
        (g["W_h2r"] * WSCALE).astype(FP8_NP),
        (g["W_h2h"] * WSCALE).astype(FP8_NP),
    ])
    wb = _gate_major([
        g["W_i2z"].astype(BF16_NP),
        g["W_h2z"].astype(BF16_NP),
        (g["W_i2h"] * PSCALE).astype(BF16_NP),
    ])
    b = np.concatenate([g["b_i2r"] * PSCALE, g["b_i2z"],
                        g["b_i2h"] * PSCALE]).astype(np.float32)
    bias_b = np.ascontiguousarray(np.broadcast_to(b, (128, 3 * H)))
    xt8_all = np.ascontiguousarray(x.T * ASCALE).astype(FP8_NP)
    ht8_all = np.ascontiguousarray(h.T * ASCALE).astype(FP8_NP)
    xtb_all = x.T.astype(BF16_NP, order="C")
    htb_all = h.T.astype(BF16_NP, order="C")
    CH = 8
    in_maps = []
    for c in range(N_CORES):
        sl = slice(c * BL, (c + 1) * BL)
        in_maps.append({
            "xt8": _chunk_major(xt8_all[:, sl], CH),
            "ht8": _chunk_major(ht8_all[:, sl], CH),
            "xtb": _chunk_major(xtb_all[:, sl], CH),
            "htb": _chunk_major(htb_all[:, sl], CH),
            "hbl": np.ascontiguousarray(h[sl].astype(BF16_NP)),
            "w8": w8,
            "wb": wb,
            "bias": bias_b,
        })
    return in_maps


_RUNNERS = {}


def get_runner(reps: int = 1):
    """Build the bass module once and wrap it in a jitted 8-way shard_map,
    mirroring concourse.bass2jax.run_bass_via_pjrt but reusable across calls
    (so repeated executions don't re-trace/re-compile). reps>1 wraps the
    whole kernel in an on-device loop (for timing via amortization)."""
    if reps in _RUNNERS:
        return _RUNNERS[reps]
    import jax
    from jax.sharding import Mesh, PartitionSpec
    from jax.experimental.shard_map import shard_map
    from concourse.bass2jax import (_bass_exec_p, install_neuronx_cc_hook,
                                    partition_id_tensor)

    nc = build_nc(reps)
    install_neuronx_cc_hook()

    partition_name = (nc.partition_id_tensor.name
                      if nc.partition_id_tensor else None)
    in_names, out_names, out_avals, zero_outs = [], [], [], []
    for alloc in nc.m.functions[0].allocations:
        if not isinstance(alloc, mybir.MemoryLocationSet):
            continue
        name = alloc.memorylocations[0].name
        if alloc.kind == "ExternalInput":
            if name != partition_name:
                in_names.append(name)
        elif alloc.kind == "ExternalOutput":
            out_names.append(name)
            shape = tuple(alloc.tensor_shape)
            dtype = mybir.dt.np(alloc.dtype)
            out_avals.append(jax.core.ShapedArray(shape, dtype))
            zero_outs.append(np.zeros(shape, dtype))
    all_names = in_names + out_names
    if partition_name is not None:
        all_names = all_names + [partition_name]
    all_names = tuple(all_names)
    n_in, n_out = len(in_names), len(out_names)

    def _body(*args):
        operands = list(args)
        if partition_name is not None:
            operands.append(partition_id_tensor())
        outs = _bass_exec_p.bind(
            *operands,
            out_avals=tuple(out_avals),
            in_names=all_names,
            out_names=tuple(out_names),
            lowering_input_output_aliases=(),
            sim_require_finite=True,
            sim_require_nnan=True,
            nc=nc,
        )
        return tuple(outs)

    devices = jax.devices()[:N_CORES]
    mesh = Mesh(np.asarray(devices), ("core",))
    sharded = jax.jit(
        shard_map(_body, mesh=mesh,
                  in_specs=(PartitionSpec("core"),) * (n_in + n_out),
                  out_specs=(PartitionSpec("core"),) * n_out,
                  check_rep=False),
        donate_argnums=tuple(range(n_in, n_in + n_out)),
        keep_unused=True,
    )
    _RUNNERS[reps] = (sharded, in_names, out_names, zero_outs)
    return _RUNNERS[reps]


def run_on_device(in_maps):
    sharded, in_names, out_names, zero_outs = get_runner()
    concat_in = [np.concatenate([m[n] for m in in_maps], axis=0)
                 for n in in_names]
    concat_zero = [np.zeros((N_CORES * z.shape[0], *z.shape[1:]), z.dtype)
                   for z in zero_outs]
    outs = sharded(*concat_in, *concat_zero)
    return {n: np.asarray(o) for n, o in zip(out_names, outs)}


_NC = None


def kernel(**inputs):
    """Full-input entry point: shard, run on 8 NeuronCores, gather."""
    global _NC
    from concourse._compat import axon_active
    in_maps = prep_in_maps(inputs)
    if axon_active():
        return run_on_device(in_maps)["out"].astype(np.float32)
    from concourse.bass_utils import run_bass_kernel_spmd
    if _NC is None:
        _NC = build_nc(1)
    res = run_bass_kernel_spmd(_NC, in_maps, core_ids=list(range(N_CORES)))
    return np.concatenate([res.results[c]["out"] for c in range(N_CORES)],
                          axis=0).astype(np.float32)
